# revision 12
# baseline (speedup 1.0000x reference)
"""Trainium2 Bass kernel for nn_MHA_2688649527670.

Reference computes, per batch b and head h:
    Q = x Wq_h^T, K = x Wk_h^T, V = x Wv_h^T          ([S, D] each)
    Z = softmax_over_d( (Q K^T / sqrt(D)) V )

There is NO softmax between Q K^T and V, so the chain is associative:
    (Q K^T) V = x * (Wq_h^T Wk_h G Wv_h^T) / sqrt(D),   G = x^T x   ([D, D])

This collapses the O(S^2 D) attention into a [D,D] weight-chain plus one
[S,D]x[D,D] matmul per head, followed by softmax over the model dim (free
axis).

Sharding: data parallel over batch (4) x tensor parallel over head-groups
(2 groups of 4 heads) = 8 cores, no collectives.

Schedule (v2):
  - x DMA first (4 splits on the sync queue), then wv, wq, wk: the critical
    path is input-bus-bound, so x (which gates G) goes first and wv (which
    gates R) leads the weights.
  - chain: P0T_h = Wk_h^T Wq_h; R_h = G_scaled Wv_h^T (per-head, right after
    G); M_h = P0T_h^T R_h.  The 1/sqrt(D) scale rides on the G psum->sbuf
    copy (scalar engine).
  - finals: per 128-row chunk, xT transpose is interleaved right before its
    y matmul; softmax uses bf16 exp outputs so the DVE sum runs in 16-bit
    mode.  Engine split per chunk: DVE max+sum+recip, scalar 4 exps,
    gpsimd the normalize multiply; psum->sbuf transpose copies rotate over
    gpsimd/vector/scalar to keep the pipeline balanced.

fp32 everywhere in the matmul chain (bf16 operands anywhere in the chain
were measured at 0.7%-6% output error); bf16 only after exp(), where
values are in [0,1] and the 2e-2 gate leaves 50x margin.
"""

import numpy as np

import concourse.bass as bass
import concourse.bacc as bacc
import concourse.mybir as mybir
import concourse.tile as tile
from concourse.bass_utils import run_bass_kernel_spmd
from concourse.masks import make_identity

B, S, D, H = 4, 2048, 128, 8
P = 128
HPC = H // 2          # heads per core (tensor parallel over 2 head groups)
NCH = S // P          # 16 s-chunks of 128 rows
N_CORES = 8
SCALE = 1.0 / float(np.sqrt(D))
F32 = mybir.dt.float32
BF16 = mybir.dt.bfloat16

_PROG = None  # cached compiled Bass program (same SPMD program for all cores)

# which engine evacuates chunk i's xT transpose out of PSUM
_COPY_ENG = ["gpsimd", "vector", "gpsimd", "scalar"] * 4


def _build_program():
    nc = bacc.Bacc("TRN2", target_bir_lowering=False, debug=False,
                   num_devices=N_CORES)

    x_d = nc.dram_tensor("x", [S, D], F32, kind="ExternalInput")
    wq_d = nc.dram_tensor("wq", [HPC * D, D], F32, kind="ExternalInput")
    wk_d = nc.dram_tensor("wk", [HPC * D, D], F32, kind="ExternalInput")
    wv_d = nc.dram_tensor("wv", [HPC * D, D], F32, kind="ExternalInput")
    out_d = nc.dram_tensor("out", [HPC, S, D], F32, kind="ExternalOutput")

    with tile.TileContext(nc) as tc:
        with (
            tc.tile_pool(name="const", bufs=1) as const,
            tc.tile_pool(name="chain", bufs=2) as chain,
            tc.tile_pool(name="work", bufs=3) as work,
            tc.tile_pool(name="ps_t", bufs=2, space="PSUM") as ps_t,
            tc.tile_pool(name="ps_g", bufs=1, space="PSUM") as ps_g,
            tc.tile_pool(name="ps_c", bufs=2, space="PSUM") as ps_c,
            tc.tile_pool(name="ps_y", bufs=3, space="PSUM") as ps_y,
        ):
            # ---- input DMAs: x first (it gates G which gates everything),
            # then wv (gates R), then wq/wk (gate P0T). All on sync queue. ----
            x_sb = const.tile([P, NCH, D], F32, tag="x_sb")
            x_view = x_d.ap().rearrange("(n p) c -> p n c", p=P)
            for q in range(4):
                nc.sync.dma_start(x_sb[:, q * 4:(q + 1) * 4, :],
                                  x_view[:, q * 4:(q + 1) * 4, :])

            w_sb = {}
            for nm, wd in (("wv", wv_d), ("wq", wq_d), ("wk", wk_d)):
                t = const.tile([P, HPC, D], F32, tag=f"{nm}_sb", name=f"{nm}_sb")
                nc.sync.dma_start(t, wd.ap().rearrange("(h p) c -> p h c", p=P))
                w_sb[nm] = t

            ident = const.tile([P, P], F32, tag="ident")
            make_identity(nc, ident)

            # ---- G = x^T x (accumulated over 16 s-chunks as they arrive),
            # then the weight chain M_h = (Wk_h^T Wq_h)^T G_s Wv_h^T.
            # The whole chain is high-priority: its matmuls and PSUM
            # evacuations gate the first final, so they must preempt the xT
            # transpose traffic in the scheduler's ready heap. ----
            with tc.high_priority():
                g_ps = ps_g.tile([P, P], F32, tag="g_ps")
                for i in range(NCH):
                    nc.tensor.matmul(g_ps, lhsT=x_sb[:, i, :],
                                     rhs=x_sb[:, i, :],
                                     start=(i == 0), stop=(i == NCH - 1))
                # fold the 1/sqrt(D) chain scale into the psum evacuation
                g_sb = const.tile([P, P], F32, tag="g_sb")
                nc.scalar.mul(g_sb, g_ps, SCALE)

                # all four heads of each chain stage share one PSUM bank;
                # per-head copies keep the next stage's head-h matmul off
                # the full-stage barrier
                # lives in the chain pool's slot rotation (wvt->p0t->r->m),
                # NOT in ps_t: sharing ps_t's tag would chain the xT pair
                # transposes behind the wv DMA via slot reuse
                wvt_ps = ps_c.tile([P, HPC, P], F32, tag="c_ps")
                wvt_sb = const.tile([P, HPC, D], F32, tag="wvt_sb")
                for h in range(HPC):
                    nc.tensor.transpose(wvt_ps[:, h, :], w_sb["wv"][:, h, :],
                                        ident)
                    nc.vector.tensor_copy(wvt_sb[:, h, :], wvt_ps[:, h, :])

                p0t_ps = ps_c.tile([P, HPC, P], F32, tag="c_ps")
                p0t_sb = chain.tile([P, HPC, D], F32, tag="p0t_sb", bufs=1)
                for h in range(HPC):
                    nc.tensor.matmul(p0t_ps[:, h, :], lhsT=w_sb["wk"][:, h, :],
                                     rhs=w_sb["wq"][:, h, :])
                    nc.vector.tensor_copy(p0t_sb[:, h, :], p0t_ps[:, h, :])

                r_ps = ps_c.tile([P, HPC, P], F32, tag="c_ps")
                r_sb = const.tile([P, HPC, D], F32, tag="r_sb")
                for h in range(HPC):
                    nc.tensor.matmul(r_ps[:, h, :], lhsT=g_sb,
                                     rhs=wvt_sb[:, h, :])
                    nc.vector.tensor_copy(r_sb[:, h, :], r_ps[:, h, :])

                m_ps = ps_c.tile([P, HPC, P], F32, tag="c_ps")
                m_all = const.tile([P, HPC, D], F32, tag="m_all")
                for h in range(HPC):
                    nc.tensor.matmul(m_ps[:, h, :], lhsT=p0t_sb[:, h, :],
                                     rhs=r_sb[:, h, :])
                    nc.vector.tensor_copy(m_all[:, h, :], m_ps[:, h, :])
            m_flat = m_all[:].rearrange("p h d -> p (h d)")

            # ---- finals: per chunk, xT transpose then Y = x @ M (N=512),
            # softmax over d per head.  Transposes run in pairs into one
            # PSUM tile; the [p,256] evacuation alternates vector/scalar
            # (gpsimd cannot read PSUM). ----
            xT_sb = const.tile([P, NCH, D], F32, tag="xT_sb")
            for i in range(NCH):
                if i % 4 == 0:
                    # transposes in quads: fewer PSUM slot reuses means fewer
                    # in-order PE stalls on evacuation copies
                    tp = ps_t.tile([P, 4, P], F32, tag="tp")
                    for j in range(4):
                        nc.tensor.transpose(tp[:, j, :], x_sb[:, i + j, :],
                                            ident)
                    nc.vector.tensor_copy(xT_sb[:, i:i + 4, :], tp)

                y_ps = ps_y.tile([P, HPC * D], F32, tag="y_ps")
                nc.tensor.matmul(y_ps, lhsT=xT_sb[:, i, :], rhs=m_flat)

                negmax = work.tile([P, HPC], F32, tag="negmax")
                nc.vector.reduce_max(
                    out=negmax,
                    in_=y_ps[:].rearrange("p (h d) -> p h d", h=HPC),
                    axis=mybir.AxisListType.X, negate=True)

                e_sb = work.tile([P, HPC, D], BF16, tag="e_sb")
                for h in range(HPC):
                    nc.scalar.activation(
                        e_sb[:, h, :], y_ps[:, h * D:(h + 1) * D],
                        mybir.ActivationFunctionType.Exp,
                        bias=negmax[:, h:h + 1], scale=1.0)

                sums = work.tile([P, HPC], BF16, tag="sums")
                with nc.allow_low_precision("softmax sum of 128 bf16 terms; "
                                            "output gate is 2e-2"):
                    nc.vector.reduce_sum(out=sums, in_=e_sb,
                                         axis=mybir.AxisListType.X)
                rsum = work.tile([P, HPC], F32, tag="rsum")
                nc.vector.reciprocal(rsum, sums)

                o_sb = work.tile([P, HPC, D], F32, tag="o_sb", bufs=4)
                nc.gpsimd.tensor_tensor(
                    o_sb, e_sb, rsum[:, :, None].to_broadcast((P, HPC, D)),
                    mybir.AluOpType.mult)
                # one DMA per chunk: DRAM walked (s, h, c) to match SBUF (p, h, c)
                nc.sync.dma_start(
                    out_d.ap()[:, i * P:(i + 1) * P, :].rearrange("h s c -> s h c"),
                    o_sb)

    nc.compile()
    return nc


def _get_program():
    global _PROG
    if _PROG is None:
        _PROG = _build_program()
    return _PROG


def _make_in_maps(x, W_q, W_k, W_v):
    in_maps = []
    for core in range(N_CORES):
        b, hg = core // 2, core % 2
        sl = slice(hg * HPC * D, (hg + 1) * HPC * D)
        in_maps.append({
            "x": np.ascontiguousarray(x[b]),
            "wq": np.ascontiguousarray(W_q[sl]),
            "wk": np.ascontiguousarray(W_k[sl]),
            "wv": np.ascontiguousarray(W_v[sl]),
        })
    return in_maps


def run(x, W_q, W_k, W_v, trace=False, **spmd_kwargs):
    """Run on 8 NeuronCores; returns (Z, BassKernelResults)."""
    nc = _get_program()
    in_maps = _make_in_maps(np.asarray(x, np.float32), np.asarray(W_q, np.float32),
                            np.asarray(W_k, np.float32), np.asarray(W_v, np.float32))
    res = run_bass_kernel_spmd(nc, in_maps, core_ids=list(range(N_CORES)),
                               trace=trace, **spmd_kwargs)
    Z = np.empty((B, H, S, D), np.float32)
    for core in range(N_CORES):
        b, hg = core // 2, core % 2
        Z[b, hg * HPC:(hg + 1) * HPC] = np.asarray(res.results[core]["out"])
    return Z, res


def kernel(x, W_q, W_k, W_v):
    Z, _ = run(x, W_q, W_k, W_v, trace=False)
    return Z


# revision 15
# speedup vs baseline: 1.1417x; 1.1417x over previous
"""Trainium2 Bass kernel for nn_MHA_2688649527670.

Reference computes, per batch b and head h:
    Q = x Wq_h^T, K = x Wk_h^T, V = x Wv_h^T          ([S, D] each)
    Z = softmax_over_d( (Q K^T / sqrt(D)) V )

There is NO softmax between Q K^T and V, so the chain is associative:
    (Q K^T) V = x * (Wq_h^T Wk_h G Wv_h^T) / sqrt(D),   G = x^T x   ([D, D])

This collapses the O(S^2 D) attention into a [D,D] weight-chain plus one
[S,D]x[D,D] matmul per head, followed by softmax over the model dim (free
axis).

Sharding: data parallel over batch (4) x tensor parallel over head-groups
(2 groups of 4 heads) = 8 cores, no collectives.

Schedule (v2):
  - x DMA first (4 splits on the sync queue), then wv, wq, wk: the critical
    path is input-bus-bound, so x (which gates G) goes first and wv (which
    gates R) leads the weights.
  - chain: P0T_h = Wk_h^T Wq_h; R_h = G_scaled Wv_h^T (per-head, right after
    G); M_h = P0T_h^T R_h.  The 1/sqrt(D) scale rides on the G psum->sbuf
    copy (scalar engine).
  - finals: per 128-row chunk, xT transpose is interleaved right before its
    y matmul; softmax uses bf16 exp outputs so the DVE sum runs in 16-bit
    mode.  Engine split per chunk: DVE max+sum+recip, scalar 4 exps,
    gpsimd the normalize multiply; psum->sbuf transpose copies rotate over
    gpsimd/vector/scalar to keep the pipeline balanced.

fp32 everywhere in the matmul chain (bf16 operands anywhere in the chain
were measured at 0.7%-6% output error); bf16 only after exp(), where
values are in [0,1] and the 2e-2 gate leaves 50x margin.
"""

import numpy as np

import concourse.bass as bass
import concourse.bacc as bacc
import concourse.mybir as mybir
import concourse.tile as tile
from concourse.bass_utils import run_bass_kernel_spmd
from concourse.masks import make_identity

B, S, D, H = 4, 2048, 128, 8
P = 128
HPC = H // 2          # heads per core (tensor parallel over 2 head groups)
NCH = S // P          # 16 s-chunks of 128 rows
N_CORES = 8
SCALE = 1.0 / float(np.sqrt(D))
F32 = mybir.dt.float32
BF16 = mybir.dt.bfloat16

_PROG = None  # cached compiled Bass program (same SPMD program for all cores)

# which engine evacuates chunk i's xT transpose out of PSUM
_COPY_ENG = ["gpsimd", "vector", "gpsimd", "scalar"] * 4


def _build_program():
    nc = bacc.Bacc("TRN2", target_bir_lowering=False, debug=False,
                   num_devices=N_CORES)

    x_d = nc.dram_tensor("x", [S, D], F32, kind="ExternalInput")
    wq_d = nc.dram_tensor("wq", [HPC * D, D], F32, kind="ExternalInput")
    wk_d = nc.dram_tensor("wk", [HPC * D, D], F32, kind="ExternalInput")
    wv_d = nc.dram_tensor("wv", [HPC * D, D], F32, kind="ExternalInput")
    out_d = nc.dram_tensor("out", [HPC, S, D], F32, kind="ExternalOutput")

    with tile.TileContext(nc) as tc:
        with (
            tc.tile_pool(name="const", bufs=1) as const,
            tc.tile_pool(name="chain", bufs=2) as chain,
            tc.tile_pool(name="work", bufs=3) as work,
            tc.tile_pool(name="ps_t", bufs=1, space="PSUM") as ps_t,
            tc.tile_pool(name="ps_g", bufs=1, space="PSUM") as ps_g,
            tc.tile_pool(name="ps_c", bufs=2, space="PSUM") as ps_c,
            tc.tile_pool(name="ps_y", bufs=4, space="PSUM") as ps_y,
        ):
            # ---- input DMAs: x first (it gates G which gates everything),
            # then wv (gates R), then wq/wk (gate P0T). All on sync queue. ----
            x_sb = const.tile([P, NCH, D], F32, tag="x_sb")
            x_view = x_d.ap().rearrange("(n p) c -> p n c", p=P)
            for q in range(4):
                nc.sync.dma_start(x_sb[:, q * 4:(q + 1) * 4, :],
                                  x_view[:, q * 4:(q + 1) * 4, :])

            w_sb = {}
            for nm, wd in (("wv", wv_d), ("wq", wq_d), ("wk", wk_d)):
                t = const.tile([P, HPC, D], F32, tag=f"{nm}_sb", name=f"{nm}_sb")
                nc.sync.dma_start(t, wd.ap().rearrange("(h p) c -> p h c", p=P))
                w_sb[nm] = t

            ident = const.tile([P, P], F32, tag="ident")
            make_identity(nc, ident)

            # ---- G = x^T x (accumulated over 16 s-chunks as they arrive),
            # then the weight chain M_h = (Wk_h^T Wq_h)^T G_s Wv_h^T.
            # The whole chain is high-priority: its matmuls and PSUM
            # evacuations gate the first final, so they must preempt the xT
            # transpose traffic in the scheduler's ready heap. ----
            with tc.high_priority():
                g_ps = ps_g.tile([P, P], F32, tag="g_ps")
                for i in range(NCH):
                    nc.tensor.matmul(g_ps, lhsT=x_sb[:, i, :],
                                     rhs=x_sb[:, i, :],
                                     start=(i == 0), stop=(i == NCH - 1))
                # fold the 1/sqrt(D) chain scale into the psum evacuation
                g_sb = const.tile([P, P], F32, tag="g_sb")
                nc.scalar.mul(g_sb, g_ps, SCALE)

                # all four heads of each chain stage share one PSUM bank;
                # per-head copies keep the next stage's head-h matmul off
                # the full-stage barrier
                # lives in the chain pool's slot rotation (wvt->p0t->r->m),
                # NOT in ps_t: sharing ps_t's tag would chain the xT pair
                # transposes behind the wv DMA via slot reuse
                wvt_ps = ps_c.tile([P, HPC, P], F32, tag="c_ps")
                wvt_sb = const.tile([P, HPC, D], F32, tag="wvt_sb")
                for h in range(HPC):
                    nc.tensor.transpose(wvt_ps[:, h, :], w_sb["wv"][:, h, :],
                                        ident)
                    nc.vector.tensor_copy(wvt_sb[:, h, :], wvt_ps[:, h, :])

                p0t_ps = ps_c.tile([P, HPC, P], F32, tag="c_ps")
                p0t_sb = chain.tile([P, HPC, D], F32, tag="p0t_sb", bufs=1)
                for h in range(HPC):
                    nc.tensor.matmul(p0t_ps[:, h, :], lhsT=w_sb["wk"][:, h, :],
                                     rhs=w_sb["wq"][:, h, :])
                    nc.vector.tensor_copy(p0t_sb[:, h, :], p0t_ps[:, h, :])

                r_ps = ps_c.tile([P, HPC, P], F32, tag="c_ps")
                r_sb = const.tile([P, HPC, D], F32, tag="r_sb")
                for h in range(HPC):
                    nc.tensor.matmul(r_ps[:, h, :], lhsT=g_sb,
                                     rhs=wvt_sb[:, h, :])
                    nc.vector.tensor_copy(r_sb[:, h, :], r_ps[:, h, :])

                m_ps = ps_c.tile([P, HPC, P], F32, tag="c_ps")
                m_all = const.tile([P, HPC, D], F32, tag="m_all")
                for h in range(HPC):
                    nc.tensor.matmul(m_ps[:, h, :], lhsT=p0t_sb[:, h, :],
                                     rhs=r_sb[:, h, :])
                    nc.vector.tensor_copy(m_all[:, h, :], m_ps[:, h, :])
            m_flat = m_all[:].rearrange("p h d -> p (h d)")

            # ---- finals: per chunk, xT transpose then Y = x @ M (N=512),
            # softmax over d per head.  Transposes run in pairs into one
            # PSUM tile; the [p,256] evacuation alternates vector/scalar
            # (gpsimd cannot read PSUM). ----
            xT_sb = const.tile([P, NCH, D], F32, tag="xT_sb")
            for i in range(NCH):
                if i % 4 == 0:
                    # transposes in quads: fewer PSUM slot reuses means fewer
                    # in-order PE stalls; evacuation on scalar (idle in
                    # phase 2) so DVE keeps only chain copies + reductions
                    tp = ps_t.tile([P, 4, P], F32, tag="tp")
                    for j in range(4):
                        nc.tensor.transpose(tp[:, j, :], x_sb[:, i + j, :],
                                            ident)
                    nc.scalar.copy(xT_sb[:, i:i + 4, :], tp)

                y_ps = ps_y.tile([P, HPC * D], F32, tag="y_ps")
                nc.tensor.matmul(y_ps, lhsT=xT_sb[:, i, :], rhs=m_flat)

                # last chunk runs as two half-head groups so the pipeline
                # drain after the final y matmul is half as deep
                groups = [(0, HPC)] if i < NCH - 1 else [(0, 2), (2, HPC)]
                for h0, h1 in groups:
                    nh = h1 - h0
                    negmax = work.tile([P, HPC], F32, tag="negmax", bufs=4)
                    nc.vector.reduce_max(
                        out=negmax[:, :nh],
                        in_=y_ps[:, h0 * D:h1 * D].rearrange(
                            "p (h d) -> p h d", h=nh),
                        axis=mybir.AxisListType.X, negate=True)

                    e_sb = work.tile([P, HPC, D], BF16, tag="e_sb", bufs=4)
                    for h in range(h0, h1):
                        nc.scalar.activation(
                            e_sb[:, h - h0, :], y_ps[:, h * D:(h + 1) * D],
                            mybir.ActivationFunctionType.Exp,
                            bias=negmax[:, h - h0:h - h0 + 1], scale=1.0)

                    sums = work.tile([P, HPC], BF16, tag="sums", bufs=4)
                    with nc.allow_low_precision("softmax sum of 128 bf16 "
                                                "terms; output gate is 2e-2"):
                        nc.vector.reduce_sum(out=sums[:, :nh],
                                             in_=e_sb[:, :nh, :],
                                             axis=mybir.AxisListType.X)
                    rsum = work.tile([P, HPC], F32, tag="rsum", bufs=4)
                    nc.vector.reciprocal(rsum[:, :nh], sums[:, :nh])

                    o_sb = work.tile([P, HPC, D], F32, tag="o_sb", bufs=4)
                    nc.gpsimd.tensor_tensor(
                        o_sb[:, :nh, :], e_sb[:, :nh, :],
                        rsum[:, :nh, None].to_broadcast((P, nh, D)),
                        mybir.AluOpType.mult)
                    # DRAM walked (s, h, c) to match SBUF (p, h, c)
                    nc.sync.dma_start(
                        out_d.ap()[h0:h1, i * P:(i + 1) * P, :].rearrange(
                            "h s c -> s h c"),
                        o_sb[:, :nh, :])

    nc.compile()
    return nc


def _get_program():
    global _PROG
    if _PROG is None:
        _PROG = _build_program()
    return _PROG


def _make_in_maps(x, W_q, W_k, W_v):
    in_maps = []
    for core in range(N_CORES):
        b, hg = core // 2, core % 2
        sl = slice(hg * HPC * D, (hg + 1) * HPC * D)
        in_maps.append({
            "x": np.ascontiguousarray(x[b]),
            "wq": np.ascontiguousarray(W_q[sl]),
            "wk": np.ascontiguousarray(W_k[sl]),
            "wv": np.ascontiguousarray(W_v[sl]),
        })
    return in_maps


def run(x, W_q, W_k, W_v, trace=False, **spmd_kwargs):
    """Run on 8 NeuronCores; returns (Z, BassKernelResults)."""
    nc = _get_program()
    in_maps = _make_in_maps(np.asarray(x, np.float32), np.asarray(W_q, np.float32),
                            np.asarray(W_k, np.float32), np.asarray(W_v, np.float32))
    res = run_bass_kernel_spmd(nc, in_maps, core_ids=list(range(N_CORES)),
                               trace=trace, **spmd_kwargs)
    Z = np.empty((B, H, S, D), np.float32)
    for core in range(N_CORES):
        b, hg = core // 2, core % 2
        Z[b, hg * HPC:(hg + 1) * HPC] = np.asarray(res.results[core]["out"])
    return Z, res


def kernel(x, W_q, W_k, W_v):
    Z, _ = run(x, W_q, W_k, W_v, trace=False)
    return Z
